# revision 1
# baseline (speedup 1.0000x reference)
"""MultiHeadAttention TRN2 kernel.

Full inputs -> shard over 8 NeuronCores -> full outputs (context, avg_attn).

Sharding: core = (batch b, query-half qh).  Each core computes its own
Q/K/V projections (K/V duplicated between the two cores of a batch),
attention for all 16 heads over its 512-query shard, the output
projection, and the head-averaged attention map.

Host-side preprocessing:
  - key_padding_mask compaction: only the unmasked key/value rows are
    shipped (padded to a fixed S_PAD=640 >> max unmasked count), which
    halves the S-dimension work.  Masked positions of avg_attn are
    exactly 0 in the reference (exp(-1e9) == 0 in fp32), so the host
    scatters the compact result back and leaves zeros elsewhere.
  - inputs/weights are transposed + cast to fp16 (PE runs fp16 at full
    rate with fp32 PSUM accumulation).

Device dataflow (per core), everything on the attention path in the
TRANSPOSED orientation [s, q] (host untransposes ctxT / avgT):
  qT/kT pair-layout big tiles [128=head-pair features, pair, q|s]
  scoresT[s,q] = k_pair.T @ q_pair        (K=64 partition slices)
  A = exp(scale*scoresT + maskbias[s])    (ACT; pad rows -> exactly 0)
  out_u[65,q] = V_aug[s,65].T @ A         (ones column -> row 64 = r)
  invr = 1/r (DVE); invr_bc = partition broadcast (GPSIMD)
  out = out_u[0:64] * invr_bc (GPSIMD; odd head DMA-shifted to
    partitions 64:128 of the pair tile)
  An = A * invr_bc (one DVE op, stride-0 bcast); avg += An (DVE)
  ctxT = WoT.T @ outT (+bo), contraction split 2x so the first half
    runs as PE filler while late heads stream through ACT/DVE.

Work is spread so no engine exceeds the PE roofline: exp on ACT,
normalize+avg on DVE, reciprocal on DVE, broadcasts + out-normalize +
ctx-half copies on the otherwise idle GPSIMD/Pool engine.  V projection
and the split output projection interleave into the attention phase as
PE filler; weight DMAs are kt-chunked (contiguous rows) so projections
start as soon as the first chunk lands.
"""

import sys

if "/opt/trn_rl_repo" not in sys.path:
    sys.path.insert(0, "/opt/trn_rl_repo")

from contextlib import ExitStack

import numpy as np

import concourse.bass as bass
import concourse.mybir as mybir
import concourse.tile as tile
from concourse import bacc
from concourse.bass_utils import run_bass_kernel_spmd

F16 = mybir.dt.float16
F32 = mybir.dt.float32

B, Q, S, H, NH = 4, 1024, 1024, 1024, 16
HD = H // NH  # 64
QSH = Q // 2  # per-core query shard
S_PAD = 640  # compacted+padded key length (P[Binom(1024,.5) > 640] ~ 0)
N_CORES = 8
SCALE = 1.0 / float(np.sqrt(HD))
MASK_NEG = -30000.0  # exp(x + MASK_NEG) == 0 exactly in fp32/f16


def build_nc(h=H, qsh=QSH, s_pad=S_PAD, reps=1):
    nh = h // HD
    np_ = nh // 2  # head pairs
    kt_n = h // 128  # contraction tiles
    st_n = s_pad // 128

    nc = bacc.Bacc("TRN2", target_bir_lowering=False)

    xqT = nc.dram_tensor("xqT", [h, qsh], F16, kind="ExternalInput")
    xkT = nc.dram_tensor("xkT", [h, s_pad], F16, kind="ExternalInput")
    xvT = nc.dram_tensor("xvT", [h, s_pad], F16, kind="ExternalInput")
    wqT = nc.dram_tensor("wqT", [h, h], F16, kind="ExternalInput")
    wkT = nc.dram_tensor("wkT", [h, h], F16, kind="ExternalInput")
    wvT = nc.dram_tensor("wvT", [h, h], F16, kind="ExternalInput")
    woT = nc.dram_tensor("woT", [h, h], F16, kind="ExternalInput")
    # consts32: [:, 0:3*kt_n] = bq|bk|bo tables, [:, 3*kt_n:] = mask bias cols
    consts32_d = nc.dram_tensor("consts32", [128, 3 * kt_n + st_n], F32,
                                kind="ExternalInput")
    # consts16: [0, 0:h] = v bias row, [0, h:h+s_pad] = valid01 row
    consts16_d = nc.dram_tensor("consts16", [1, h + s_pad], F16,
                                kind="ExternalInput")
    ctxT = nc.dram_tensor("ctxT", [h, qsh], F16, kind="ExternalOutput")
    # f16 and unscaled: the host divides by NH during reassembly
    avgT = nc.dram_tensor("avgT", [s_pad, qsh], F16, kind="ExternalOutput")

    with tile.TileContext(nc) as tc, ExitStack() as ctx:
        pwc = ctx.enter_context(tc.tile_pool(name="wc", bufs=2 * kt_n))
        pw = ctx.enter_context(tc.tile_pool(name="w", bufs=1))
        px = ctx.enter_context(tc.tile_pool(name="x", bufs=3))
        pq = ctx.enter_context(tc.tile_pool(name="qb", bufs=1))
        pk = ctx.enter_context(tc.tile_pool(name="kb", bufs=1))
        pv = ctx.enter_context(tc.tile_pool(name="vsb", bufs=st_n))
        pa = ctx.enter_context(tc.tile_pool(name="A", bufs=4))
        pan = ctx.enter_context(tc.tile_pool(name="An", bufs=2))
        pavg = ctx.enter_context(tc.tile_pool(name="avga", bufs=1))
        pout = ctx.enter_context(tc.tile_pool(name="outT", bufs=kt_n))
        ph1 = ctx.enter_context(tc.tile_pool(name="ch1", bufs=kt_n))
        pinv = ctx.enter_context(tc.tile_pool(name="inv", bufs=3))
        pbc = ctx.enter_context(tc.tile_pool(name="bc", bufs=4))
        pstg = ctx.enter_context(tc.tile_pool(name="stg", bufs=3))
        prow = ctx.enter_context(tc.tile_pool(name="row", bufs=1))
        pctx = ctx.enter_context(tc.tile_pool(name="ctxe", bufs=4))
        paf = ctx.enter_context(tc.tile_pool(name="af", bufs=1))
        pps = ctx.enter_context(tc.tile_pool(name="psp", bufs=2, space="PSUM"))
        psc = ctx.enter_context(tc.tile_pool(name="pssc", bufs=3, space="PSUM"))
        pou = ctx.enter_context(tc.tile_pool(name="psou", bufs=3, space="PSUM"))

        for _rep in range(reps):
            # ---- input DMAs, in consumption order; weights kt-chunked ----
            # first weight chunk ahead of xq: the first Q-proj matmul waits
            # on both, and xq is the longer transfer
            wq_c, wk_c, wv_c = [], [], []
            wc0 = pwc.tile([128, h], F16, tag="wc", name="wq0")
            nc.sync.dma_start(wc0[:], wqT[0:128, :])
            wq_c.append(wc0)
            # xq in two halves so the first Q-proj matmuls start after half 1
            xq_h = []
            for i in range(2):
                t = px.tile([128, kt_n // 2, qsh], F16, tag="xqh",
                            name=f"xq{i}")
                nc.sync.dma_start(
                    t[:], xqT[i * (h // 2):(i + 1) * (h // 2), :]
                    .rearrange("(t p) c -> p t c", p=128))
                xq_h.append(t)
            # constants next: not needed until the first bias-add (~8us)
            c32 = prow.tile([128, 3 * kt_n + st_n], F32, tag="c32")
            nc.sync.dma_start(c32[:], consts32_d[:])
            c16 = prow.tile([1, h + s_pad], F16, tag="c16")
            nc.sync.dma_start(c16[:], consts16_d[:])
            for kt in range(1, kt_n):
                wc = pwc.tile([128, h], F16, tag="wc", name=f"wq{kt}")
                nc.sync.dma_start(wc[:], wqT[kt * 128:(kt + 1) * 128, :])
                wq_c.append(wc)
            xk = px.tile([128, kt_n, s_pad], F16, tag="xin", name="xk")
            nc.sync.dma_start(xk[:], xkT.rearrange("(t p) c -> p t c", p=128))
            for kt in range(kt_n):
                wc = pwc.tile([128, h], F16, tag="wc", name=f"wk{kt}")
                nc.sync.dma_start(wc[:], wkT[kt * 128:(kt + 1) * 128, :])
                wk_c.append(wc)
            xv = px.tile([128, kt_n, s_pad], F16, tag="xin", name="xv")
            nc.sync.dma_start(xv[:], xvT.rearrange("(t p) c -> p t c", p=128))
            for kt in range(kt_n):
                wc = pwc.tile([128, h], F16, tag="wc", name=f"wv{kt}")
                nc.sync.dma_start(wc[:], wvT[kt * 128:(kt + 1) * 128, :])
                wv_c.append(wc)
            wo = pw.tile([128, kt_n, h], F16, tag="w", name="wo")
            nc.sync.dma_start(wo[:], woT.rearrange("(t p) c -> p t c", p=128))

            bvr = c16[0:1, 0:h]
            val01 = c16[0:1, h:h + s_pad]

            def bq_t(p):
                return c32[:, p:p + 1]

            def bk_t(p):
                return c32[:, kt_n + p:kt_n + p + 1]

            def bo_t(p):
                return c32[:, 2 * kt_n + p:2 * kt_n + p + 1]

            def mk_t(st):
                return c32[:, 3 * kt_n + st:3 * kt_n + st + 1]

            # ---- big SBUF tiles ----
            onesr16 = prow.tile([1, 128], F16, tag="ones16")
            nc.vector.memset(onesr16[:], 1.0)
            qbig = pq.tile([128, np_, qsh], F16, tag="qb", name="qbig")
            kbig = pk.tile([128, np_, s_pad], F16, tag="kb", name="kbig")
            v_sb = [pv.tile([128, nh, HD + 1], F16, tag="vsb", name=f"v{st}")
                    for st in range(st_n)]
            avg_acc = pavg.tile([128, st_n * qsh], F16, tag="avga")
            nc.gpsimd.memset(avg_acc[:], 0.0)
            for st in range(st_n):
                nc.gpsimd.memset(v_sb[st][:, :, HD:HD + 1], 1.0)
            out_sb = [pout.tile([128, qsh], F16, tag="outT", name=f"o{p}")
                      for p in range(np_)]
            ctx_h1 = [ph1.tile([128, qsh], F32, tag="ch1", name=f"h1{p}")
                      for p in range(kt_n)]

            def qproj(p):
                ps = pps.tile([128, qsh], F32, tag="psp", name=f"psq{p}")
                for kt in range(kt_n):
                    nc.tensor.matmul(ps[:], wq_c[kt][:, p * 128:(p + 1) * 128],
                                     xq_h[kt // (kt_n // 2)][:, kt % (kt_n // 2), :],
                                     start=(kt == 0), stop=(kt == kt_n - 1))
                nc.vector.tensor_scalar_add(qbig[:, p, :], ps[:], bq_t(p)[:])

            def kproj(p):
                for c0 in range(0, s_pad, 512):
                    cw = min(512, s_pad - c0)
                    ps = pps.tile([128, qsh], F32, tag="psp", name=f"psk{p}")
                    for kt in range(kt_n):
                        nc.tensor.matmul(ps[:, 0:cw],
                                         wk_c[kt][:, p * 128:(p + 1) * 128],
                                         xk[:, kt, c0:c0 + cw],
                                         start=(kt == 0), stop=(kt == kt_n - 1))
                    nc.scalar.activation(kbig[:, p, c0:c0 + cw], ps[:, 0:cw],
                                         mybir.ActivationFunctionType.Identity,
                                         bias=bk_t(p)[:])

            def vproj(st, c):
                c0 = c * 512
                ps = pps.tile([128, qsh], F32, tag="psp", name=f"psv{st}{c}")
                for kt in range(kt_n):
                    nc.tensor.matmul(ps[:], xv[:, kt, st * 128:(st + 1) * 128],
                                     wv_c[kt][:, c0:c0 + 512],
                                     start=(kt == 0), stop=False)
                nc.tensor.matmul(ps[:], val01[0:1, st * 128:(st + 1) * 128],
                                 bvr[0:1, c0:c0 + 512], start=False, stop=True)
                # f32 psum -> f16 needs a compute engine; GPSIMD can't read
                # PSUM on TRN2, so this stays on ACT
                nc.scalar.copy(
                    v_sb[st][:, c * 8:(c + 1) * 8, 0:HD],
                    ps[:].rearrange("p (a b) -> p a b", a=8))

            kt_h1 = 5  # ctx half-1 contraction depth (pairs 0-4)

            def ohalf1(ot):
                ps = pps.tile([128, qsh], F32, tag="psp", name=f"ph1{ot}")
                for kt in range(kt_h1):
                    nc.tensor.matmul(
                        ps[:], wo[:, kt, ot * 128:(ot + 1) * 128], out_sb[kt][:],
                        start=(kt == 0), stop=(kt == kt_h1 - 1))
                # psum -> sbuf spill on ACT (Pool and DMA can't read PSUM)
                nc.scalar.copy(ctx_h1[ot][:], ps[:])

            # ---- projections (DMA-paced), then attention ----
            for p in range(np_):
                qproj(p)
            for p in range(np_):
                kproj(p)

            # PE filler work slotted between scores_h and attnV_h.
            # v c=1 chunks feed heads 8-15; ctx half-1 feeds the tail.
            fillers = {
                3: [lambda: vproj(0, 1)],
                4: [lambda: vproj(1, 1)],
                5: [lambda: vproj(2, 1)],
                6: [lambda: vproj(3, 1)],
                7: [lambda: vproj(4, 1)],
                10: [lambda: ohalf1(0), lambda: ohalf1(1)],
                11: [lambda: ohalf1(2)],
                12: [lambda: ohalf1(3)],
                13: [lambda: ohalf1(4), lambda: ohalf1(5)],
                15: [lambda: ohalf1(6)],
                14: [lambda: ohalf1(7)],
            }

            def emit_norm_avg(a_big, invr_bc):
                an_big = pan.tile([128, st_n, qsh], F16, tag="An")
                for st in range(st_n):
                    nc.vector.tensor_tensor(an_big[:, st, :], a_big[:, st, :],
                                            invr_bc[:],
                                            op=mybir.AluOpType.mult)
                nc.vector.tensor_tensor(
                    avg_acc[:], avg_acc[:],
                    an_big[:].rearrange("p a b -> p (a b)"),
                    op=mybir.AluOpType.add)

            # head 14 (pair-7 even half) processed LAST: its out-normalize
            # writes out_sb[7][0:64] directly (no partition-shift DMA on the
            # critical path into the ctx second half)
            head_order = list(range(nh - 2)) + [nh - 1, nh - 2]
            na_queue = []
            for pos, hd_i in enumerate(head_order):
                p, half = divmod(hd_i, 2)
                r0, r1 = half * 64, half * 64 + 64
                a_big = pa.tile([128, st_n, qsh], F16, tag="A", name=f"A{hd_i}")
                for st in range(st_n):
                    ps = psc.tile([128, qsh], F32, tag="pssc")
                    nc.tensor.matmul(
                        ps[:], kbig[r0:r1, p, st * 128:(st + 1) * 128],
                        qbig[r0:r1, p, :], start=True, stop=True)
                    nc.scalar.activation(a_big[:, st, :], ps[:],
                                         mybir.ActivationFunctionType.Exp,
                                         bias=mk_t(st)[:], scale=SCALE)
                ou = pou.tile([128, qsh], F32, tag="psou", name=f"ou{hd_i}")
                if hd_i == 0:
                    # v c=0 chunks interleave with head-0 attnV accumulation
                    for st in range(st_n):
                        vproj(st, 0)
                        nc.tensor.matmul(ou[0:HD + 1, :],
                                         v_sb[st][:, hd_i, 0:HD + 1],
                                         a_big[:, st, :],
                                         start=(st == 0), stop=(st == st_n - 1))
                else:
                    for f in fillers.get(hd_i, []):
                        f()
                    for st in range(st_n):
                        nc.tensor.matmul(ou[0:HD + 1, :],
                                         v_sb[st][:, hd_i, 0:HD + 1],
                                         a_big[:, st, :],
                                         start=(st == 0), stop=(st == st_n - 1))
                # softmax denominator r = ou row 64 -> 1/r -> broadcast
                # (recip emitted BEFORE the deferred norm/avg of the previous
                # head so it doesn't queue behind 2.8us of DVE bulk work)
                invr = pinv.tile([1, qsh], F16, tag="inv")
                with nc.allow_low_precision(reason="1/r in f16"):
                    nc.vector.reciprocal(invr[:], ou[HD:HD + 1, :])
                # broadcast 1/r to 128 partitions: ones[1,128].T @ invr[1,q]
                # (PE rank-1; the gpsimd partition_broadcast extended inst
                # returned wrong upper partitions on hardware)
                psb = pou.tile([128, qsh], F32, tag="psou", name=f"psb{hd_i}")
                nc.tensor.matmul(psb[:], onesr16[:], invr[:],
                                 start=True, stop=True)
                invr_bc = pbc.tile([128, qsh], F16, tag="bc", name=f"bc{hd_i}")
                nc.scalar.copy(invr_bc[:], psb[:])
                # out-normalize next (DVE: Pool can't read PSUM); it gates
                # the output projection
                if half == 0:
                    nc.vector.tensor_tensor(out_sb[p][0:64, :], ou[0:64, :],
                                            invr_bc[0:64, :],
                                            op=mybir.AluOpType.mult)
                else:
                    stg = pstg.tile([64, qsh], F16, tag="stg")
                    nc.vector.tensor_tensor(stg[:], ou[0:64, :],
                                            invr_bc[0:64, :],
                                            op=mybir.AluOpType.mult)
                    nc.sync.dma_start(out_sb[p][64:128, :], stg[:])
                # normalize A + accumulate avg, deferred two heads so each
                # head's reciprocal never queues behind DVE bulk work; the
                # final heads drain eagerly so no backlog trails attention
                na_queue.append((a_big, invr_bc))
                pops = 2 if pos in (nh - 3, nh - 2) else \
                    (1 if len(na_queue) > 2 else 0)
                for _ in range(min(pops, len(na_queue))):
                    emit_norm_avg(*na_queue.pop(0))
            # the final head's norm/avg drains AFTER the ctx combines below
            # (DVE is in-order; only the avgT DMA depends on it)

            # ---- output projection, second half + combine ----
            # combines alternate Pool/DVE (both idle-ish by now) and psums
            # alternate pools (attention pools are free) so PE streams all
            # 24 matmuls back-to-back while the combines drain in parallel
            opools = [(pps, "psp"), (psc, "pssc"), (pou, "psou")]
            for ot in range(kt_n):
                pool, ptag = opools[ot % 3]
                ps = pool.tile([128, qsh], F32, tag=ptag, name=f"pso{ot}")
                for kt in range(kt_h1, kt_n):
                    nc.tensor.matmul(
                        ps[:], wo[:, kt, ot * 128:(ot + 1) * 128], out_sb[kt][:],
                        start=(kt == kt_h1), stop=(kt == kt_n - 1))
                ctx_e = pctx.tile([128, qsh], F16, tag="ctxe")
                with nc.allow_low_precision(reason="f16 ctx within tolerance"):
                    nc.vector.scalar_tensor_tensor(
                        ctx_e[:], ps[:], bo_t(ot)[:], ctx_h1[ot][:],
                        op0=mybir.AluOpType.add, op1=mybir.AluOpType.add)
                nc.sync.dma_start(ctxT[ot * 128:(ot + 1) * 128, :], ctx_e[:])
            while na_queue:
                emit_norm_avg(*na_queue.pop(0))
            # avg_attn out AFTER every head's accumulate is emitted (the
            # tile deps only cover writes emitted before the DMA).  Raw f16
            # head-sum; host scales by 1/NH.  ACT DGE queue, parallel to
            # the ctx stream on SP.
            nc.scalar.dma_start(avgT.rearrange("(t p) c -> p t c", p=128),
                                avg_acc[:].rearrange("p (t c) -> p t c",
                                                     t=st_n))

    nc.compile()
    return nc


_NC_CACHE = {}


def _get_nc():
    if "nc" not in _NC_CACHE:
        _NC_CACHE["nc"] = build_nc()
    return _NC_CACHE["nc"]


def make_in_maps(query, key, value, key_padding_mask,
                 Wq, bq, Wk, bk, Wv, bv, Wo, bo):
    query = np.asarray(query, np.float32)
    key = np.asarray(key, np.float32)
    value = np.asarray(value, np.float32)
    mask = np.asarray(key_padding_mask, bool)
    wqT16 = np.ascontiguousarray(np.asarray(Wq, np.float32).T.astype(np.float16))
    wkT16 = np.ascontiguousarray(np.asarray(Wk, np.float32).T.astype(np.float16))
    wvT16 = np.ascontiguousarray(np.asarray(Wv, np.float32).T.astype(np.float16))
    woT16 = np.ascontiguousarray(np.asarray(Wo, np.float32).T.astype(np.float16))
    kt_n = H // 128
    st_n = S_PAD // 128
    bvr = np.asarray(bv, np.float32).reshape(1, H).astype(np.float16)

    idx_list = [np.nonzero(~mask[b])[0] for b in range(B)]
    in_maps = []
    for core in range(N_CORES):
        b, qh = divmod(core, 2)
        idx = idx_list[b]
        se = len(idx)
        xq = query[b, qh * QSH:(qh + 1) * QSH, :]
        xk = np.zeros((S_PAD, H), np.float16)
        xk[:se] = key[b, idx, :].astype(np.float16)
        xv = np.zeros((S_PAD, H), np.float16)
        xv[:se] = value[b, idx, :].astype(np.float16)
        c16 = np.zeros((1, H + S_PAD), np.float16)
        c16[0, :H] = bvr[0]
        c16[0, H:H + se] = 1.0
        c32 = np.zeros((128, 3 * kt_n + st_n), np.float32)
        for i, bias in enumerate((bq, bk, bo)):
            c32[:, i * kt_n:(i + 1) * kt_n] = \
                np.asarray(bias, np.float32).reshape(kt_n, 128).T
        sidx = np.arange(S_PAD).reshape(st_n, 128).T  # [128, st]
        c32[:, 3 * kt_n:][sidx >= se] = MASK_NEG
        in_maps.append({
            "xqT": np.ascontiguousarray(xq.T.astype(np.float16)),
            "xkT": np.ascontiguousarray(xk.T),
            "xvT": np.ascontiguousarray(xv.T),
            "wqT": wqT16, "wkT": wkT16, "wvT": wvT16, "woT": woT16,
            "consts32": c32, "consts16": c16,
        })
    return in_maps, idx_list


def assemble(results, idx_list):
    context = np.empty((B, Q, H), np.float32)
    avg = np.zeros((B, Q, S), np.float32)
    for core in range(N_CORES):
        b, qh = divmod(core, 2)
        rows = slice(qh * QSH, (qh + 1) * QSH)
        context[b, rows, :] = results[core]["ctxT"].T
        idx = idx_list[b]
        # advanced-index dims (b, idx) move to the front; avgT is [s, q],
        # f16 and unscaled (the device ships the raw head-sum)
        avg[b, rows, idx] = \
            results[core]["avgT"][:len(idx), :].astype(np.float32) * (1.0 / NH)
    return context, avg


def _numpy_fallback(query, key, value, key_padding_mask,
                    Wq, bq, Wk, bk, Wv, bv, Wo, bo):
    """Exact fp32 reference path, used only if the mask compaction budget
    would overflow (cannot happen for the spec'd input distribution)."""
    q = (query @ Wq.T + bq).reshape(B, Q, NH, HD).transpose(0, 2, 1, 3)
    k = (key @ Wk.T + bk).reshape(B, S, NH, HD).transpose(0, 2, 1, 3)
    v = (value @ Wv.T + bv).reshape(B, S, NH, HD).transpose(0, 2, 1, 3)
    s = np.einsum("bhqd,bhsd->bhqs", q, k) / np.sqrt(HD)
    s = np.where(np.asarray(key_padding_mask, bool)[:, None, None, :], -1e9, s)
    s = s - s.max(-1, keepdims=True)
    a = np.exp(s)
    a /= a.sum(-1, keepdims=True)
    out = np.einsum("bhqs,bhsd->bhqd", a, v)
    out = out.transpose(0, 2, 1, 3).reshape(B, Q, H)
    return (out @ Wo.T + bo).astype(np.float32), \
        a.mean(axis=1).astype(np.float32)


def kernel(query, key, value, key_padding_mask,
           Wq, bq, Wk, bk, Wv, bv, Wo, bo):
    assert query.shape == (B, Q, H) and key.shape == (B, S, H)
    mask = np.asarray(key_padding_mask, bool)
    if max((~mask[b]).sum() for b in range(B)) > S_PAD:
        return _numpy_fallback(query, key, value, key_padding_mask,
                               Wq, bq, Wk, bk, Wv, bv, Wo, bo)
    in_maps, idx_list = make_in_maps(query, key, value, key_padding_mask,
                                     Wq, bq, Wk, bk, Wv, bv, Wo, bo)
    res = run_bass_kernel_spmd(_get_nc(), in_maps,
                               core_ids=list(range(N_CORES)))
    return assemble(res.results, idx_list)

